# revision 26
# baseline (speedup 1.0000x reference)
"""Trainium2 Bass kernel for nn_Blast: out = x @ (W0 + 1 bias^T) + bias
where W0 block (i_in, i_out) = Vt[i] @ diag(S[o,i]) @ U[o].

z-factorized algorithm (per core, 256 tokens, all-bf16 streams):
  z[(i,r), t]   = sum_p Vt'[i,p,r] * xT[i-block p, t]   (32 MMs, M=17)
  mid[(o,r), t] = sum_(i,r') W2[(i,r'),(o,r)] * z       (16 MMs, W2 = S
                                                         scattered, host-built)
  out[t, oq]    = sum_r mid[(o,r), t] * U''[o,r,q]      (K=64 row-strip MMs,
                                                         block-diag U64 pairs)

Layouts: z lives in 4 PSUM groups x 4 col-strip slots (i -> group i//4,
rows 32*(i%4)..+17); mid in 4 PSUM groups x 4 slots (o -> group o//4, rows
32*(o%4)..+18).  Vt' has a 17th all-ones column so z row 32m+16 is the
block-colsum of x; W2 routes those to every mid rowsum row (32j+16).
Phase B fuses o-block pairs: one K=64 matmul per pair against a shipped
block-diagonal U64 (zero rows cover the mid padding), so each [128,512]
PSUM bank needs only one engine copy to SBUF.

Bias trick: out = x@W0 + (rowsum(x)+1)*bias.  Each z bank is opened by a
K=1 matmul writing 1/16 everywhere, so the 16 colsum rows sum to
rowsum+1 in the mid rowsum row; z rows 32m+17 stay at exactly 1/16 and
W2 (value 16 at one of them) turns that into a constant-1 mid row
(32j+17).  U'' row 16 = bias (x rowsum+1), row 17 cancels the 1/16
pollution of the rank rows: -(1/16)*sum_r (sum_i S[o,i,r]) U[o,r,:].

Everything streamed in bf16 (PSUM accumulates f32); host pre-transposes
x, pre-builds W2/Vt'/U'', and upcasts the bf16 output back to f32.
Sharding: pure data-parallel over the 2048 tokens (8 cores x 256).
"""

import numpy as np

IN_DIM = 4096
OUT_DIM = 4096
BLOCK = 256
RANK = 16
B_IN = 16
B_OUT = 16
N_CORES = 8
TOK = 2048
TPC = TOK // N_CORES          # 256 tokens per core
RA = RANK + 1                 # 17: rank cols + colsum col per chunk
KU = RANK + 2                 # 18: used rows of U'' / mid per o-block
NCHUNK = IN_DIM // 128        # 32 K-chunks
NWARM = 6                     # PE warmup matmuls
CINIT = 1.0 / 16.0            # z bank init constant

_CACHE = {}

# test.py toggles; harness never touches these
TRACE = False
TRACE_DIR = None
LAST_RESULTS = None


def build_program():
    import concourse.mybir as mybir
    from concourse import bacc
    from concourse.tile import TileContext

    f32 = mybir.dt.float32
    bf16 = mybir.dt.bfloat16

    nc = bacc.Bacc(trn_type="TRN2")
    # const blobs: c0 = [wseed | konst row], vt, w2 early; u64 queued on the
    # sync ring AFTER the x batches so it never competes with the x stream
    C0 = BLOCK + 384
    xt_d = nc.dram_tensor("xt", (128, NCHUNK * TPC), bf16, kind="ExternalInput")
    c0_d = nc.dram_tensor("c0", (128, C0), bf16, kind="ExternalInput")
    vt_d = nc.dram_tensor("vt", (128, NCHUNK * RA), bf16, kind="ExternalInput")
    c2_d = nc.dram_tensor("c2", (128, 4 * 512), bf16, kind="ExternalInput")
    u64_d = nc.dram_tensor("u64", (128, 4 * 512), bf16, kind="ExternalInput")
    out_d = nc.dram_tensor("out", (TPC, OUT_DIM), bf16, kind="ExternalOutput")

    with TileContext(nc) as tc:
        from contextlib import ExitStack

        with ExitStack() as ctx:
            consts = ctx.enter_context(tc.tile_pool(name="consts", bufs=1))
            xpool = ctx.enter_context(tc.tile_pool(name="xpool", bufs=1))
            zsb = ctx.enter_context(tc.tile_pool(name="zsb", bufs=1))
            midsb = ctx.enter_context(tc.tile_pool(name="midsb", bufs=1))
            outsb = ctx.enter_context(tc.tile_pool(name="outsb", bufs=6))
            ps_mid = ctx.enter_context(
                tc.tile_pool(name="ps_mid", bufs=1, space="PSUM")
            )

            # ---- input loads ----
            # c0 (wseed | konst) then vt on the sync ring ahead of x; c2
            # (w2 | u64) on the scalar ring in parallel
            c0_sb = consts.tile([128, C0], bf16, name="c0_sb", tag="c0_sb")
            nc.sync.dma_start(out=c0_sb[:], in_=c0_d[:])
            wsb = c0_sb[:, 0:BLOCK]
            ones_sb = c0_sb[0:1, BLOCK : BLOCK + 128]
            crow_sb = c0_sb[0:1, BLOCK + 128 : BLOCK + 384]
            vt_sb = consts.tile([128, NCHUNK * RA], bf16, name="vt_sb", tag="vt_sb")
            nc.sync.dma_start(out=vt_sb[:], in_=vt_d[:])

            c2_sb = consts.tile([128, 4 * 512], bf16, name="c2_sb", tag="c2_sb")
            nc.scalar.dma_start(out=c2_sb[:], in_=c2_d[:])
            w2_sb = c2_sb[:, 0 : 4 * 512]
            usb = consts.tile([128, 4 * 512], bf16, name="usb", tag="usb")

            # x^T chunk batches on the sync queue
            xbatches = []
            xslices = []
            XGRPS = [4, 8, 10, 10]
            base = 0
            for b, xg in enumerate(XGRPS):
                xb = xpool.tile([128, xg * TPC], bf16, name=f"xb{b}", tag=f"xb{b}")
                nc.sync.dma_start(
                    out=xb[:], in_=xt_d[:, base * TPC : (base + xg) * TPC]
                )
                for kk in range(xg):
                    xslices.append(xb[:, kk * TPC : (kk + 1) * TPC])
                xbatches.append(xb)
                base += xg
            # u64 drains after the x stream (sync-ring FIFO), lands ~19us,
            # needed only at phase B (~22us)
            nc.sync.dma_start(out=usb[:], in_=u64_d[:])

            # ---- PSUM z pool (+ warmup) ----
            mids_shuf = midsb.tile(
                [128, 4 * TPC], bf16, name="mids_shuf", tag="mids_shuf"
            )
            zts = []
            with tc.tile_pool(name="ps_z", bufs=1, space="PSUM") as ps_z:
                for g in range(4):
                    zt = ps_z.tile([128, TPC], f32, name=f"zp{g}", tag=f"zp{g}")
                    zts.append(zt)

                # warmups share z bank 0 (the init matmul clears it after)
                for w in range(NWARM):
                    nc.tensor.matmul(
                        zts[0][:],
                        lhsT=wsb[:, 0:128],
                        rhs=wsb[:],
                        start=True,
                        stop=True,
                        tile_position=(0, 0),
                    )

                # open z banks with CINIT everywhere (K=1 matmul)
                for g in range(4):
                    nc.tensor.matmul(
                        zts[g][:],
                        lhsT=ones_sb,
                        rhs=crow_sb,
                        start=True,
                        stop=False,
                        tile_position=(0, 0),
                    )

                # ---- phase Z: z[(i,r),t] accumulation, 2 chunks per i ----
                zcopies = []
                mixmm = []
                for i in range(B_IN):
                    g, mp = i // 4, i % 4
                    for h in range(2):
                        c = 2 * i + h
                        nc.tensor.matmul(
                            zts[g][32 * mp : 32 * mp + RA, :],
                            lhsT=vt_sb[:, RA * c : RA * (c + 1)],
                            rhs=xslices[c],
                            start=False,
                            stop=(mp == 3 and h == 1),
                            tile_position=(0, 32 * mp),
                            skip_group_check=True,
                        )
                    if mp == 3:
                        # group g complete: stage to SBUF (bf16) and mix
                        zc = zsb.tile([128, TPC], bf16, name=f"zsb{g}", tag=f"zsb{g}")
                        if g % 2 == 0:
                            nc.vector.tensor_copy(zc[:], zts[g][:])
                        else:
                            nc.scalar.copy(zc[:], zts[g][:])
                        zcopies.append(zc)

                # ---- mix: mid[(o,r),t] = W2^T z, into 4 slot-layout banks ----
                midp = []
                for t in range(4):
                    mp_t = ps_mid.tile(
                        [128, TPC], f32, name=f"midp{t}", tag=f"midp{t}"
                    )
                    midp.append(mp_t)
                for g in range(4):
                    for t in range(4):
                        nc.tensor.matmul(
                            midp[t][:],
                            lhsT=w2_sb[:, 512 * g + 128 * t : 512 * g + 128 * (t + 1)],
                            rhs=zcopies[g][:],
                            start=(g == 0),
                            stop=(g == 3),
                            tile_position=(0, 0),
                        )

            # ---- mid to SBUF (bf16), full tiles, partition-preserving ----
            for t in range(4):
                dst = mids_shuf[:, t * TPC : (t + 1) * TPC]
                if t % 2 == 0:
                    nc.vector.tensor_copy(dst, midp[t][:])
                else:
                    nc.scalar.copy(dst, midp[t][:])

            # ---- phase B: out tiles [128 tok, 256 q], K=18, 4 row-strips ----
            ps_out = ctx.enter_context(
                tc.tile_pool(name="ps_out", bufs=4, space="PSUM")
            )
            for tt in range(TPC // 128):
                for t in range(4):
                    osb_t = outsb.tile(
                        [128, 4 * BLOCK], bf16, name="osb", tag="osb"
                    )
                    for p in range(2):
                        po = ps_out.tile([128, 2 * BLOCK], f32, name="po", tag="po")
                        nc.tensor.matmul(
                            po[:],
                            lhsT=mids_shuf[
                                64 * p : 64 * p + 64,
                                t * TPC + tt * 128 : t * TPC + (tt + 1) * 128,
                            ],
                            rhs=usb[
                                64 * p : 64 * p + 64,
                                t * 2 * BLOCK : (t + 1) * 2 * BLOCK,
                            ],
                            start=True,
                            stop=True,
                            tile_position=(64 * p, 0),
                        )
                        eng = [nc.vector.tensor_copy, nc.scalar.copy][p]
                        eng(osb_t[:, 2 * p * BLOCK : 2 * (p + 1) * BLOCK], po[:])
                    nc.scalar.dma_start(
                        out=out_d[
                            tt * 128 : (tt + 1) * 128,
                            t * 4 * BLOCK : (t + 1) * 4 * BLOCK,
                        ],
                        in_=osb_t[:],
                    )

    nc.compile()
    return nc


def prep_inputs(x, S, U, Vt, bias):
    """Host-side layout prep. Returns per-core input maps."""
    import ml_dtypes

    bf = ml_dtypes.bfloat16
    x = np.asarray(x, dtype=np.float32)
    S = np.asarray(S, dtype=np.float32)
    U = np.asarray(U, dtype=np.float32)
    Vt = np.asarray(Vt, dtype=np.float32)
    bias = np.asarray(bias, dtype=np.float32)

    xt = x.reshape(TOK, IN_DIM).T.astype(bf)          # (4096, 2048)
    xt_i = np.ascontiguousarray(
        xt.reshape(NCHUNK, 128, TOK).transpose(1, 0, 2)
    )  # (128, 32, 2048)

    # vt_sb[p, 17c + r] = Vt[i, 128h+p, r] (c = 2i+h), col 16 = ones
    vt_aug = np.ones((B_IN, BLOCK, RA), np.float32)
    vt_aug[:, :, :RANK] = Vt
    vt_host = np.ascontiguousarray(
        vt_aug.reshape(B_IN * 2, 128, RA)  # (c, p, r)
        .transpose(1, 0, 2)                # (p, c, r)
        .reshape(128, NCHUNK * RA)
        .astype(bf)
    )

    # W2[(g=i//4, 32*(i%4)+r'), (t=o//4, 32*(o%4)+rr)] block layout:
    #   r'<16, rr=r':  S[o, i, r']
    #   r'=16, rr=16:  1            (colsum rows -> rowsum row)
    #   r'=17 (g=0,i%4=0 only), rr=17: 16   (CINIT row -> const-1 row)
    w2 = np.zeros((4, 128, 4, 128), np.float32)  # (g, zrow, t, midcol)
    for i in range(B_IN):
        g, mp = i // 4, i % 4
        for o in range(B_OUT):
            t, j = o // 4, o % 4
            for r in range(RANK):
                w2[g, 32 * mp + r, t, 32 * j + r] = S[o, i, r]
            w2[g, 32 * mp + RANK, t, 32 * j + RANK] = 1.0
    for o in range(B_OUT):
        t, j = o // 4, o % 4
        w2[0, RANK + 1, t, 32 * j + RANK + 1] = 16.0
    w2_host = np.ascontiguousarray(
        w2.transpose(1, 0, 2, 3).reshape(128, 4 * 512).astype(bf)
    )

    # U'' rows: [U (16); bias (1); comp (1)]
    bias_row = bias.reshape(B_OUT, 1, BLOCK)
    s_sum = S.sum(axis=1)  # (B_OUT, RANK): sum_i S[o,i,r]
    comp_row = -(CINIT) * np.einsum("or,orq->oq", s_sum, U)[:, None, :]
    u_aug = np.concatenate([U, bias_row, comp_row], axis=1)  # (16, 18, 256)
    # u64 block-diagonal: u64[64p+32s+r, 512t+256s+q] = U''[4t+2p+s, r, q]
    u_host = np.zeros((128, 4 * 2 * BLOCK), np.float32)
    uv = u_host.reshape(2, 2, 32, 4, 2, BLOCK)  # (p, s_row, r, t, s_col, q)
    ua = u_aug.reshape(4, 2, 2, KU, BLOCK)      # (t, p, s, r, q)
    for s in range(2):
        uv[:, s, :KU, :, s, :] = ua[:, :, s].transpose(1, 2, 0, 3)

    rng = np.random.default_rng(0)
    wseed = rng.standard_normal((128, BLOCK), dtype=np.float32)

    c0 = np.zeros((128, BLOCK + 384), np.float32)
    c0[:, :BLOCK] = wseed
    c0[0, BLOCK : BLOCK + 128] = 1.0
    c0[0, BLOCK + 128 : BLOCK + 384] = CINIT
    c0 = np.ascontiguousarray(c0.astype(bf))

    c2 = np.ascontiguousarray(w2_host)
    u64 = np.ascontiguousarray(u_host.astype(bf))

    in_maps = []
    for c in range(N_CORES):
        in_maps.append(
            {
                "xt": np.ascontiguousarray(
                    xt_i[:, :, c * TPC : (c + 1) * TPC].reshape(128, -1)
                ),
                "c0": c0,
                "vt": vt_host,
                "c2": c2,
                "u64": u64,
            }
        )
    return in_maps


def kernel(x, S, U, Vt, bias):
    global LAST_RESULTS
    from concourse.bass_utils import run_bass_kernel_spmd

    if "nc" not in _CACHE:
        _CACHE["nc"] = build_program()
    nc = _CACHE["nc"]

    in_maps = prep_inputs(x, S, U, Vt, bias)
    res = run_bass_kernel_spmd(
        nc, in_maps, list(range(N_CORES)), trace=TRACE, tmpdir=TRACE_DIR
    )
    LAST_RESULTS = res
    out = np.concatenate(
        [res.results[c]["out"].astype(np.float32) for c in range(N_CORES)], axis=0
    )
    return out.reshape(2, TOK // 2, OUT_DIM)


# revision 27
# speedup vs baseline: 1.0283x; 1.0283x over previous
"""Trainium2 Bass kernel for nn_Blast: out = x @ (W0 + 1 bias^T) + bias
where W0 block (i_in, i_out) = Vt[i] @ diag(S[o,i]) @ U[o].

z-factorized algorithm (per core, 256 tokens, all-bf16 streams):
  z[(i,r), t]   = sum_p Vt'[i,p,r] * xT[i-block p, t]   (32 MMs, M=17)
  mid[(o,r), t] = sum_(i,r') W2[(i,r'),(o,r)] * z       (16 MMs, W2 = S
                                                         scattered, host-built)
  out[t, oq]    = sum_r mid[(o,r), t] * U''[o,r,q]      (K=64 row-strip MMs,
                                                         block-diag U64 pairs)

Layouts: z lives in 4 PSUM groups x 4 col-strip slots (i -> group i//4,
rows 32*(i%4)..+17); mid in 4 PSUM groups x 4 slots (o -> group o//4, rows
32*(o%4)..+18).  Vt' has a 17th all-ones column so z row 32m+16 is the
block-colsum of x; W2 routes those to every mid rowsum row (32j+16).
Phase B fuses o-block pairs: one K=64 matmul per pair against a shipped
block-diagonal U64 (zero rows cover the mid padding), so each [128,512]
PSUM bank needs only one engine copy to SBUF.

Bias trick: out = x@W0 + (rowsum(x)+1)*bias.  Each z bank is opened by a
K=1 matmul writing 1/16 everywhere, so the 16 colsum rows sum to
rowsum+1 in the mid rowsum row; z rows 32m+17 stay at exactly 1/16 and
W2 (value 16 at one of them) turns that into a constant-1 mid row
(32j+17).  U'' row 16 = bias (x rowsum+1), row 17 cancels the 1/16
pollution of the rank rows: -(1/16)*sum_r (sum_i S[o,i,r]) U[o,r,:].

Everything streamed in bf16 (PSUM accumulates f32); host pre-transposes
x, pre-builds W2/Vt'/U'', and upcasts the bf16 output back to f32.
Sharding: pure data-parallel over the 2048 tokens (8 cores x 256).
"""

import numpy as np

IN_DIM = 4096
OUT_DIM = 4096
BLOCK = 256
RANK = 16
B_IN = 16
B_OUT = 16
N_CORES = 8
TOK = 2048
TPC = TOK // N_CORES          # 256 tokens per core
RA = RANK + 1                 # 17: rank cols + colsum col per chunk
KU = RANK + 2                 # 18: used rows of U'' / mid per o-block
NCHUNK = IN_DIM // 128        # 32 K-chunks
NWARM = 6                     # PE warmup matmuls
CINIT = 1.0 / 16.0            # z bank init constant

_CACHE = {}

# test.py toggles; harness never touches these
TRACE = False
TRACE_DIR = None
LAST_RESULTS = None


def build_program():
    import concourse.mybir as mybir
    from concourse import bacc
    from concourse.tile import TileContext

    f32 = mybir.dt.float32
    bf16 = mybir.dt.bfloat16

    nc = bacc.Bacc(trn_type="TRN2")
    # const blobs: c0 = [wseed | konst row], vt, w2 early; u64 queued on the
    # sync ring AFTER the x batches so it never competes with the x stream
    C0 = BLOCK + 384
    xt_d = nc.dram_tensor("xt", (128, NCHUNK * TPC), bf16, kind="ExternalInput")
    c0_d = nc.dram_tensor("c0", (128, C0), bf16, kind="ExternalInput")
    vt_d = nc.dram_tensor("vt", (128, NCHUNK * RA), bf16, kind="ExternalInput")
    c2_d = nc.dram_tensor("c2", (128, 4 * 512), bf16, kind="ExternalInput")
    u64_d = nc.dram_tensor("u64", (128, 4 * 512), bf16, kind="ExternalInput")
    out_d = nc.dram_tensor("out", (TPC, OUT_DIM), bf16, kind="ExternalOutput")

    with TileContext(nc) as tc:
        from contextlib import ExitStack

        with ExitStack() as ctx:
            consts = ctx.enter_context(tc.tile_pool(name="consts", bufs=1))
            xpool = ctx.enter_context(tc.tile_pool(name="xpool", bufs=1))
            zsb = ctx.enter_context(tc.tile_pool(name="zsb", bufs=1))
            midsb = ctx.enter_context(tc.tile_pool(name="midsb", bufs=1))
            outsb = ctx.enter_context(tc.tile_pool(name="outsb", bufs=6))
            ps_mid = ctx.enter_context(
                tc.tile_pool(name="ps_mid", bufs=1, space="PSUM")
            )

            # ---- input loads ----
            # c0 (wseed | konst) then vt on the sync ring ahead of x; c2
            # (w2 | u64) on the scalar ring in parallel
            c0_sb = consts.tile([128, C0], bf16, name="c0_sb", tag="c0_sb")
            nc.sync.dma_start(out=c0_sb[:], in_=c0_d[:])
            wsb = c0_sb[:, 0:BLOCK]
            ones_sb = c0_sb[0:1, BLOCK : BLOCK + 128]
            crow_sb = c0_sb[0:1, BLOCK + 128 : BLOCK + 384]
            vt_sb = consts.tile([128, NCHUNK * RA], bf16, name="vt_sb", tag="vt_sb")
            nc.sync.dma_start(out=vt_sb[:], in_=vt_d[:])

            c2_sb = consts.tile([128, 4 * 512], bf16, name="c2_sb", tag="c2_sb")
            nc.scalar.dma_start(out=c2_sb[:], in_=c2_d[:])
            w2_sb = c2_sb[:, 0 : 4 * 512]
            usb = consts.tile([128, 4 * 512], bf16, name="usb", tag="usb")

            # x^T chunk batches on the sync queue
            xbatches = []
            xslices = []
            XGRPS = [4, 8, 10, 10]
            base = 0
            for b, xg in enumerate(XGRPS):
                xb = xpool.tile([128, xg * TPC], bf16, name=f"xb{b}", tag=f"xb{b}")
                nc.sync.dma_start(
                    out=xb[:], in_=xt_d[:, base * TPC : (base + xg) * TPC]
                )
                for kk in range(xg):
                    xslices.append(xb[:, kk * TPC : (kk + 1) * TPC])
                xbatches.append(xb)
                base += xg
            # u64 drains after the x stream (sync-ring FIFO), lands ~19us,
            # needed only at phase B (~22us)
            nc.sync.dma_start(out=usb[:], in_=u64_d[:])

            # ---- PSUM z pool (+ warmup) ----
            mids_shuf = midsb.tile(
                [128, 4 * TPC], bf16, name="mids_shuf", tag="mids_shuf"
            )
            zts = []
            with tc.tile_pool(name="ps_z", bufs=1, space="PSUM") as ps_z:
                for g in range(4):
                    zt = ps_z.tile([128, TPC], f32, name=f"zp{g}", tag=f"zp{g}")
                    zts.append(zt)

                # warmups share z bank 0 (the init matmul clears it after)
                for w in range(NWARM):
                    nc.tensor.matmul(
                        zts[0][:],
                        lhsT=wsb[:, 0:128],
                        rhs=wsb[:],
                        start=True,
                        stop=True,
                        tile_position=(0, 0),
                    )

                # open z banks with CINIT everywhere (K=1 matmul)
                for g in range(4):
                    nc.tensor.matmul(
                        zts[g][:],
                        lhsT=ones_sb,
                        rhs=crow_sb,
                        start=True,
                        stop=False,
                        tile_position=(0, 0),
                    )

                # ---- phase Z: z[(i,r),t] accumulation, 2 chunks per i ----
                zcopies = []
                mixmm = []
                for i in range(B_IN):
                    g, mp = i // 4, i % 4
                    for h in range(2):
                        c = 2 * i + h
                        nc.tensor.matmul(
                            zts[g][32 * mp : 32 * mp + RA, :],
                            lhsT=vt_sb[:, RA * c : RA * (c + 1)],
                            rhs=xslices[c],
                            start=False,
                            stop=(mp == 3 and h == 1),
                            tile_position=(0, 32 * mp),
                            skip_group_check=True,
                        )
                    if mp == 3:
                        # group g complete: stage to SBUF (bf16) and mix
                        zc = zsb.tile([128, TPC], bf16, name=f"zsb{g}", tag=f"zsb{g}")
                        if g % 2 == 0:
                            nc.vector.tensor_copy(zc[:], zts[g][:])
                        else:
                            nc.scalar.copy(zc[:], zts[g][:])
                        zcopies.append(zc)

                # ---- mix: mid[(o,r),t] = W2^T z, into 4 slot-layout banks ----
                midp = []
                for t in range(4):
                    mp_t = ps_mid.tile(
                        [128, TPC], f32, name=f"midp{t}", tag=f"midp{t}"
                    )
                    midp.append(mp_t)
                for g in range(4):
                    for t in range(4):
                        nc.tensor.matmul(
                            midp[t][:],
                            lhsT=w2_sb[:, 512 * g + 128 * t : 512 * g + 128 * (t + 1)],
                            rhs=zcopies[g][:],
                            start=(g == 0),
                            stop=(g == 3),
                            tile_position=(0, 0),
                        )

            # ---- mid to SBUF (bf16), full tiles, partition-preserving ----
            for t in range(4):
                dst = mids_shuf[:, t * TPC : (t + 1) * TPC]
                if t % 2 == 0:
                    nc.vector.tensor_copy(dst, midp[t][:])
                else:
                    nc.scalar.copy(dst, midp[t][:])

            # ---- phase B: out tiles [128 tok, 256 q], K=18, 4 row-strips ----
            ps_out = ctx.enter_context(
                tc.tile_pool(name="ps_out", bufs=4, space="PSUM")
            )
            for tt in range(TPC // 128):
                for t in range(4):
                    osb_t = outsb.tile(
                        [128, 4 * BLOCK], bf16, name="osb", tag="osb"
                    )
                    for p in range(2):
                        po = ps_out.tile([128, 2 * BLOCK], f32, name="po", tag="po")
                        nc.tensor.matmul(
                            po[:],
                            lhsT=mids_shuf[
                                64 * p : 64 * p + 64,
                                t * TPC + tt * 128 : t * TPC + (tt + 1) * 128,
                            ],
                            rhs=usb[
                                64 * p : 64 * p + 64,
                                t * 2 * BLOCK : (t + 1) * 2 * BLOCK,
                            ],
                            start=True,
                            stop=True,
                            tile_position=(64 * p, 0),
                        )
                        eng = [nc.vector.tensor_copy, nc.scalar.copy][p]
                        eng(osb_t[:, 2 * p * BLOCK : 2 * (p + 1) * BLOCK], po[:])
                    odma = [nc.scalar.dma_start, nc.sync.dma_start][(tt * 4 + t) % 2]
                    odma(
                        out=out_d[
                            tt * 128 : (tt + 1) * 128,
                            t * 4 * BLOCK : (t + 1) * 4 * BLOCK,
                        ],
                        in_=osb_t[:],
                    )

    nc.compile()
    return nc


def prep_inputs(x, S, U, Vt, bias):
    """Host-side layout prep. Returns per-core input maps."""
    import ml_dtypes

    bf = ml_dtypes.bfloat16
    x = np.asarray(x, dtype=np.float32)
    S = np.asarray(S, dtype=np.float32)
    U = np.asarray(U, dtype=np.float32)
    Vt = np.asarray(Vt, dtype=np.float32)
    bias = np.asarray(bias, dtype=np.float32)

    xt = x.reshape(TOK, IN_DIM).T.astype(bf)          # (4096, 2048)
    xt_i = np.ascontiguousarray(
        xt.reshape(NCHUNK, 128, TOK).transpose(1, 0, 2)
    )  # (128, 32, 2048)

    # vt_sb[p, 17c + r] = Vt[i, 128h+p, r] (c = 2i+h), col 16 = ones
    vt_aug = np.ones((B_IN, BLOCK, RA), np.float32)
    vt_aug[:, :, :RANK] = Vt
    vt_host = np.ascontiguousarray(
        vt_aug.reshape(B_IN * 2, 128, RA)  # (c, p, r)
        .transpose(1, 0, 2)                # (p, c, r)
        .reshape(128, NCHUNK * RA)
        .astype(bf)
    )

    # W2[(g=i//4, 32*(i%4)+r'), (t=o//4, 32*(o%4)+rr)] block layout:
    #   r'<16, rr=r':  S[o, i, r']
    #   r'=16, rr=16:  1            (colsum rows -> rowsum row)
    #   r'=17 (g=0,i%4=0 only), rr=17: 16   (CINIT row -> const-1 row)
    w2 = np.zeros((4, 128, 4, 128), np.float32)  # (g, zrow, t, midcol)
    for i in range(B_IN):
        g, mp = i // 4, i % 4
        for o in range(B_OUT):
            t, j = o // 4, o % 4
            for r in range(RANK):
                w2[g, 32 * mp + r, t, 32 * j + r] = S[o, i, r]
            w2[g, 32 * mp + RANK, t, 32 * j + RANK] = 1.0
    for o in range(B_OUT):
        t, j = o // 4, o % 4
        w2[0, RANK + 1, t, 32 * j + RANK + 1] = 16.0
    w2_host = np.ascontiguousarray(
        w2.transpose(1, 0, 2, 3).reshape(128, 4 * 512).astype(bf)
    )

    # U'' rows: [U (16); bias (1); comp (1)]
    bias_row = bias.reshape(B_OUT, 1, BLOCK)
    s_sum = S.sum(axis=1)  # (B_OUT, RANK): sum_i S[o,i,r]
    comp_row = -(CINIT) * np.einsum("or,orq->oq", s_sum, U)[:, None, :]
    u_aug = np.concatenate([U, bias_row, comp_row], axis=1)  # (16, 18, 256)
    # u64 block-diagonal: u64[64p+32s+r, 512t+256s+q] = U''[4t+2p+s, r, q]
    u_host = np.zeros((128, 4 * 2 * BLOCK), np.float32)
    uv = u_host.reshape(2, 2, 32, 4, 2, BLOCK)  # (p, s_row, r, t, s_col, q)
    ua = u_aug.reshape(4, 2, 2, KU, BLOCK)      # (t, p, s, r, q)
    for s in range(2):
        uv[:, s, :KU, :, s, :] = ua[:, :, s].transpose(1, 2, 0, 3)

    rng = np.random.default_rng(0)
    wseed = rng.standard_normal((128, BLOCK), dtype=np.float32)

    c0 = np.zeros((128, BLOCK + 384), np.float32)
    c0[:, :BLOCK] = wseed
    c0[0, BLOCK : BLOCK + 128] = 1.0
    c0[0, BLOCK + 128 : BLOCK + 384] = CINIT
    c0 = np.ascontiguousarray(c0.astype(bf))

    c2 = np.ascontiguousarray(w2_host)
    u64 = np.ascontiguousarray(u_host.astype(bf))

    in_maps = []
    for c in range(N_CORES):
        in_maps.append(
            {
                "xt": np.ascontiguousarray(
                    xt_i[:, :, c * TPC : (c + 1) * TPC].reshape(128, -1)
                ),
                "c0": c0,
                "vt": vt_host,
                "c2": c2,
                "u64": u64,
            }
        )
    return in_maps


def kernel(x, S, U, Vt, bias):
    global LAST_RESULTS
    from concourse.bass_utils import run_bass_kernel_spmd

    if "nc" not in _CACHE:
        _CACHE["nc"] = build_program()
    nc = _CACHE["nc"]

    in_maps = prep_inputs(x, S, U, Vt, bias)
    res = run_bass_kernel_spmd(
        nc, in_maps, list(range(N_CORES)), trace=TRACE, tmpdir=TRACE_DIR
    )
    LAST_RESULTS = res
    out = np.concatenate(
        [res.results[c]["out"].astype(np.float32) for c in range(N_CORES)], axis=0
    )
    return out.reshape(2, TOK // 2, OUT_DIM)


# revision 30
# speedup vs baseline: 1.0430x; 1.0143x over previous
"""Trainium2 Bass kernel for nn_Blast: out = x @ (W0 + 1 bias^T) + bias
where W0 block (i_in, i_out) = Vt[i] @ diag(S[o,i]) @ U[o].

z-factorized algorithm (per core, 256 tokens, all-bf16 streams):
  z[(i,r), t]   = sum_p Vt'[i,p,r] * xT[i-block p, t]   (32 MMs, M=17)
  mid[(o,r), t] = sum_(i,r') W2[(i,r'),(o,r)] * z       (16 MMs, W2 = S
                                                         scattered, host-built)
  out[t, oq]    = sum_r mid[(o,r), t] * U''[o,r,q]      (K=64 row-strip MMs,
                                                         block-diag U64 pairs)

Layouts: z lives in 4 PSUM groups x 4 col-strip slots (i -> group i//4,
rows 32*(i%4)..+17); mid in 4 PSUM groups x 4 slots (o -> group o//4, rows
32*(o%4)..+18).  Vt' has a 17th all-ones column so z row 32m+16 is the
block-colsum of x; W2 routes those to every mid rowsum row (32j+16).
Phase B fuses o-block pairs: one K=64 matmul per pair against a shipped
block-diagonal U64 (zero rows cover the mid padding), so each [128,512]
PSUM bank needs only one engine copy to SBUF.

Bias trick: out = x@W0 + (rowsum(x)+1)*bias.  Each z bank is opened by a
K=1 matmul writing 1/16 everywhere, so the 16 colsum rows sum to
rowsum+1 in the mid rowsum row; z rows 32m+17 stay at exactly 1/16 and
W2 (value 16 at one of them) turns that into a constant-1 mid row
(32j+17).  U'' row 16 = bias (x rowsum+1), row 17 cancels the 1/16
pollution of the rank rows: -(1/16)*sum_r (sum_i S[o,i,r]) U[o,r,:].

Everything streamed in bf16 (PSUM accumulates f32); host pre-transposes
x, pre-builds W2/Vt'/U'', and upcasts the bf16 output back to f32.
Sharding: pure data-parallel over the 2048 tokens (8 cores x 256).
"""

import numpy as np

IN_DIM = 4096
OUT_DIM = 4096
BLOCK = 256
RANK = 16
B_IN = 16
B_OUT = 16
N_CORES = 8
TOK = 2048
TPC = TOK // N_CORES          # 256 tokens per core
RA = RANK + 1                 # 17: rank cols + colsum col per chunk
KU = RANK + 2                 # 18: used rows of U'' / mid per o-block
NCHUNK = IN_DIM // 128        # 32 K-chunks
NWARM = 6                     # PE warmup matmuls
CINIT = 1.0 / 16.0            # z bank init constant

_CACHE = {}

# test.py toggles; harness never touches these
TRACE = False
TRACE_DIR = None
LAST_RESULTS = None


def build_program():
    import concourse.mybir as mybir
    from concourse import bacc
    from concourse.tile import TileContext

    f32 = mybir.dt.float32
    bf16 = mybir.dt.bfloat16

    nc = bacc.Bacc(trn_type="TRN2")
    # const blobs: c0 = [wseed | konst row], vt, w2 early; u64 queued on the
    # sync ring AFTER the x batches so it never competes with the x stream
    C0 = BLOCK + 384
    xt_d = nc.dram_tensor("xt", (128, NCHUNK * TPC), bf16, kind="ExternalInput")
    c0_d = nc.dram_tensor("c0", (128, C0), bf16, kind="ExternalInput")
    vt_d = nc.dram_tensor("vt", (128, NCHUNK * RA), bf16, kind="ExternalInput")
    c2_d = nc.dram_tensor("c2", (128, 4 * 512), bf16, kind="ExternalInput")
    u64_d = nc.dram_tensor("u64", (128, 4 * 512), bf16, kind="ExternalInput")
    out_d = nc.dram_tensor("out", (TPC, OUT_DIM), bf16, kind="ExternalOutput")

    with TileContext(nc) as tc:
        from contextlib import ExitStack

        with ExitStack() as ctx:
            consts = ctx.enter_context(tc.tile_pool(name="consts", bufs=1))
            xpool = ctx.enter_context(tc.tile_pool(name="xpool", bufs=1))
            zsb = ctx.enter_context(tc.tile_pool(name="zsb", bufs=1))
            midsb = ctx.enter_context(tc.tile_pool(name="midsb", bufs=1))
            outsb = ctx.enter_context(tc.tile_pool(name="outsb", bufs=6))
            ps_mid = ctx.enter_context(
                tc.tile_pool(name="ps_mid", bufs=1, space="PSUM")
            )

            # ---- input loads ----
            # c0 (wseed | konst) then vt on the sync ring ahead of x; c2
            # (w2 | u64) on the scalar ring in parallel
            c0_sb = consts.tile([128, C0], bf16, name="c0_sb", tag="c0_sb")
            nc.sync.dma_start(out=c0_sb[:], in_=c0_d[:])
            wsb = c0_sb[:, 0:BLOCK]
            ones_sb = c0_sb[0:1, BLOCK : BLOCK + 128]
            crow_sb = c0_sb[0:1, BLOCK + 128 : BLOCK + 384]
            vt_sb = consts.tile([128, NCHUNK * RA], bf16, name="vt_sb", tag="vt_sb")
            nc.sync.dma_start(out=vt_sb[:], in_=vt_d[:])

            c2_sb = consts.tile([128, 4 * 512], bf16, name="c2_sb", tag="c2_sb")
            nc.scalar.dma_start(out=c2_sb[:], in_=c2_d[:])
            w2_sb = c2_sb[:, 0 : 4 * 512]
            usb = consts.tile([128, 4 * 512], bf16, name="usb", tag="usb")

            # x^T chunk batches on the sync queue
            xbatches = []
            xslices = []
            XGRPS = [4, 8, 10, 10]
            base = 0
            for b, xg in enumerate(XGRPS):
                xb = xpool.tile([128, xg * TPC], bf16, name=f"xb{b}", tag=f"xb{b}")
                nc.sync.dma_start(
                    out=xb[:], in_=xt_d[:, base * TPC : (base + xg) * TPC]
                )
                for kk in range(xg):
                    xslices.append(xb[:, kk * TPC : (kk + 1) * TPC])
                xbatches.append(xb)
                base += xg
            # u64 drains after the x stream (sync-ring FIFO), lands ~19us,
            # needed only at phase B (~22us)
            nc.sync.dma_start(out=usb[:], in_=u64_d[:])

            # ---- PSUM z pool (+ warmup) ----
            mids_shuf = midsb.tile(
                [128, 4 * TPC], bf16, name="mids_shuf", tag="mids_shuf"
            )
            zts = []
            with tc.tile_pool(name="ps_z", bufs=1, space="PSUM") as ps_z:
                for g in range(4):
                    zt = ps_z.tile([128, TPC], f32, name=f"zp{g}", tag=f"zp{g}")
                    zts.append(zt)

                # warmups share z bank 0 (the init matmul clears it after)
                for w in range(NWARM):
                    nc.tensor.matmul(
                        zts[0][:],
                        lhsT=wsb[:, 0:128],
                        rhs=wsb[:],
                        start=True,
                        stop=True,
                        tile_position=(0, 0),
                    )

                # open z banks with CINIT everywhere (K=1 matmul)
                for g in range(4):
                    nc.tensor.matmul(
                        zts[g][:],
                        lhsT=ones_sb,
                        rhs=crow_sb,
                        start=True,
                        stop=False,
                        tile_position=(0, 0),
                    )

                midp = []
                for t in range(4):
                    mp_t = ps_mid.tile(
                        [128, TPC], f32, name=f"midp{t}", tag=f"midp{t}"
                    )
                    midp.append(mp_t)

                # ---- phase Z: z[(i,r),t] accumulation, 2 chunks per i ----
                # (late dummy full-K matmuls -- cleared by the mix start --
                # keep the PE activity monitor latched into mix/phase B)
                zcopies = []
                for i in range(B_IN):
                    g, mp = i // 4, i % 4
                    for h in range(2):
                        c = 2 * i + h
                        nc.tensor.matmul(
                            zts[g][32 * mp : 32 * mp + RA, :],
                            lhsT=vt_sb[:, RA * c : RA * (c + 1)],
                            rhs=xslices[c],
                            start=False,
                            stop=(mp == 3 and h == 1),
                            tile_position=(0, 32 * mp),
                            skip_group_check=True,
                        )
                    if 10 <= i < 15:
                        nc.tensor.matmul(
                            midp[i % 4][:],
                            lhsT=wsb[:, 0:128],
                            rhs=wsb[:],
                            start=True,
                            stop=True,
                            tile_position=(0, 0),
                            skip_group_check=True,
                        )
                    if mp == 3:
                        # group g complete: stage to SBUF (bf16) and mix
                        zc = zsb.tile([128, TPC], bf16, name=f"zsb{g}", tag=f"zsb{g}")
                        if g % 2 == 0:
                            nc.vector.tensor_copy(zc[:], zts[g][:])
                        else:
                            nc.scalar.copy(zc[:], zts[g][:])
                        zcopies.append(zc)

                # ---- mix: mid[(o,r),t] = W2^T z, into 4 slot-layout banks ----
                for g in range(4):
                    for t in range(4):
                        nc.tensor.matmul(
                            midp[t][:],
                            lhsT=w2_sb[:, 512 * g + 128 * t : 512 * g + 128 * (t + 1)],
                            rhs=zcopies[g][:],
                            start=(g == 0),
                            stop=(g == 3),
                            tile_position=(0, 0),
                            skip_group_check=True,
                        )

            # ---- phase B interleaved with the mid copies (t-outer): the
            # first B group's copy rides the engine opposite its mid copy ----
            ps_out = ctx.enter_context(
                tc.tile_pool(name="ps_out", bufs=4, space="PSUM")
            )
            for t in range(4):
                dst = mids_shuf[:, t * TPC : (t + 1) * TPC]
                if t % 2 == 0:
                    nc.vector.tensor_copy(dst, midp[t][:])
                else:
                    nc.scalar.copy(dst, midp[t][:])
                for tt in range(TPC // 128):
                    osb_t = outsb.tile(
                        [128, 4 * BLOCK], bf16, name="osb", tag="osb"
                    )
                    for p in range(2):
                        po = ps_out.tile([128, 2 * BLOCK], f32, name="po", tag="po")
                        nc.tensor.matmul(
                            po[:],
                            lhsT=mids_shuf[
                                64 * p : 64 * p + 64,
                                t * TPC + tt * 128 : t * TPC + (tt + 1) * 128,
                            ],
                            rhs=usb[
                                64 * p : 64 * p + 64,
                                t * 2 * BLOCK : (t + 1) * 2 * BLOCK,
                            ],
                            start=True,
                            stop=True,
                            tile_position=(64 * p, 0),
                        )
                        eng = [nc.vector.tensor_copy, nc.scalar.copy][
                            (t + tt + p + 1) % 2
                        ]
                        eng(osb_t[:, 2 * p * BLOCK : 2 * (p + 1) * BLOCK], po[:])
                    odma = [nc.scalar.dma_start, nc.sync.dma_start][(tt * 4 + t) % 2]
                    odma(
                        out=out_d[
                            tt * 128 : (tt + 1) * 128,
                            t * 4 * BLOCK : (t + 1) * 4 * BLOCK,
                        ],
                        in_=osb_t[:],
                    )

    nc.compile()
    return nc


def prep_inputs(x, S, U, Vt, bias):
    """Host-side layout prep. Returns per-core input maps."""
    import ml_dtypes

    bf = ml_dtypes.bfloat16
    x = np.asarray(x, dtype=np.float32)
    S = np.asarray(S, dtype=np.float32)
    U = np.asarray(U, dtype=np.float32)
    Vt = np.asarray(Vt, dtype=np.float32)
    bias = np.asarray(bias, dtype=np.float32)

    xt = x.reshape(TOK, IN_DIM).T.astype(bf)          # (4096, 2048)
    xt_i = np.ascontiguousarray(
        xt.reshape(NCHUNK, 128, TOK).transpose(1, 0, 2)
    )  # (128, 32, 2048)

    # vt_sb[p, 17c + r] = Vt[i, 128h+p, r] (c = 2i+h), col 16 = ones
    vt_aug = np.ones((B_IN, BLOCK, RA), np.float32)
    vt_aug[:, :, :RANK] = Vt
    vt_host = np.ascontiguousarray(
        vt_aug.reshape(B_IN * 2, 128, RA)  # (c, p, r)
        .transpose(1, 0, 2)                # (p, c, r)
        .reshape(128, NCHUNK * RA)
        .astype(bf)
    )

    # W2[(g=i//4, 32*(i%4)+r'), (t=o//4, 32*(o%4)+rr)] block layout:
    #   r'<16, rr=r':  S[o, i, r']
    #   r'=16, rr=16:  1            (colsum rows -> rowsum row)
    #   r'=17 (g=0,i%4=0 only), rr=17: 16   (CINIT row -> const-1 row)
    w2 = np.zeros((4, 128, 4, 128), np.float32)  # (g, zrow, t, midcol)
    for i in range(B_IN):
        g, mp = i // 4, i % 4
        for o in range(B_OUT):
            t, j = o // 4, o % 4
            for r in range(RANK):
                w2[g, 32 * mp + r, t, 32 * j + r] = S[o, i, r]
            w2[g, 32 * mp + RANK, t, 32 * j + RANK] = 1.0
    for o in range(B_OUT):
        t, j = o // 4, o % 4
        w2[0, RANK + 1, t, 32 * j + RANK + 1] = 16.0
    w2_host = np.ascontiguousarray(
        w2.transpose(1, 0, 2, 3).reshape(128, 4 * 512).astype(bf)
    )

    # U'' rows: [U (16); bias (1); comp (1)]
    bias_row = bias.reshape(B_OUT, 1, BLOCK)
    s_sum = S.sum(axis=1)  # (B_OUT, RANK): sum_i S[o,i,r]
    comp_row = -(CINIT) * np.einsum("or,orq->oq", s_sum, U)[:, None, :]
    u_aug = np.concatenate([U, bias_row, comp_row], axis=1)  # (16, 18, 256)
    # u64 block-diagonal: u64[64p+32s+r, 512t+256s+q] = U''[4t+2p+s, r, q]
    u_host = np.zeros((128, 4 * 2 * BLOCK), np.float32)
    uv = u_host.reshape(2, 2, 32, 4, 2, BLOCK)  # (p, s_row, r, t, s_col, q)
    ua = u_aug.reshape(4, 2, 2, KU, BLOCK)      # (t, p, s, r, q)
    for s in range(2):
        uv[:, s, :KU, :, s, :] = ua[:, :, s].transpose(1, 2, 0, 3)

    rng = np.random.default_rng(0)
    wseed = rng.standard_normal((128, BLOCK), dtype=np.float32)

    c0 = np.zeros((128, BLOCK + 384), np.float32)
    c0[:, :BLOCK] = wseed
    c0[0, BLOCK : BLOCK + 128] = 1.0
    c0[0, BLOCK + 128 : BLOCK + 384] = CINIT
    c0 = np.ascontiguousarray(c0.astype(bf))

    c2 = np.ascontiguousarray(w2_host)
    u64 = np.ascontiguousarray(u_host.astype(bf))

    in_maps = []
    for c in range(N_CORES):
        in_maps.append(
            {
                "xt": np.ascontiguousarray(
                    xt_i[:, :, c * TPC : (c + 1) * TPC].reshape(128, -1)
                ),
                "c0": c0,
                "vt": vt_host,
                "c2": c2,
                "u64": u64,
            }
        )
    return in_maps


def kernel(x, S, U, Vt, bias):
    global LAST_RESULTS
    from concourse.bass_utils import run_bass_kernel_spmd

    if "nc" not in _CACHE:
        _CACHE["nc"] = build_program()
    nc = _CACHE["nc"]

    in_maps = prep_inputs(x, S, U, Vt, bias)
    res = run_bass_kernel_spmd(
        nc, in_maps, list(range(N_CORES)), trace=TRACE, tmpdir=TRACE_DIR
    )
    LAST_RESULTS = res
    out = np.concatenate(
        [res.results[c]["out"].astype(np.float32) for c in range(N_CORES)], axis=0
    )
    return out.reshape(2, TOK // 2, OUT_DIM)
